# revision 32
# baseline (speedup 1.0000x reference)
"""Trainium2 Bass kernel for nn_Attention_31490700214694 (sparse_attention).

Pipeline (per batch, replicating the reference exactly, incl. the raw-reshape
aliasing in msa):
  x --shuffle--> xs --1x1 conv (192->576)--> qkv --dw3x3--> qkv_dw
     --window-attention (on the raw-reshape aliased view)--> attn
     --1x1 conv (192->192)--> out --shuffle_back--> result

Sharding: 8 cores = 4 batches x 2 channel-halves.  qkv channels [0:288) of a
batch alias exactly to attention rows h' in [0,128) (288*65536 == 128*147456),
and attention output rows h'' in [0,128) alias to attn channels [0:96).  So
each core runs the full middle pipeline independently on its half; the final
1x1 proj conv is computed as a partial sum over the core's 96 attn channels
and the two partials per batch are summed on the host.  No collectives.

v2: bf16 compute + intermediates; depthwise channel-quarter packing (zero
partition waste) with dense-AP STT chains (shifted-copy trick for DVE 2x
mode) on DVE and diag-matmul slabs on TensorE; attention with dense bf16
products (ACT exp-expansion) and GpSimd offloads.
"""

import os
import sys

import numpy as np

sys.path.insert(0, "/opt/trn_rl_repo")

def _install_ntff_hook():
    """Provide antenv.axon_hooks (missing in this image) so that
    run_bass_kernel_spmd(trace=True) can capture NTFF profiles."""
    import types
    import ctypes
    import contextlib

    if "antenv.axon_hooks" in sys.modules:
        return
    so_path = os.environ.get("PJRT_LIBRARY_PATH", "/opt/axon/libaxon_pjrt.so")
    try:
        lib = ctypes.CDLL(so_path)
    except OSError:
        return
    if not hasattr(lib, "axon_start_nrt_profile"):
        return
    lib.axon_start_nrt_profile.argtypes = [
        ctypes.POINTER(ctypes.c_int64), ctypes.c_size_t]
    lib.axon_start_nrt_profile.restype = ctypes.c_int64
    lib.axon_stop_nrt_profile.argtypes = [ctypes.c_char_p]
    lib.axon_stop_nrt_profile.restype = ctypes.c_int64

    @contextlib.contextmanager
    def _hook(output_dir, device_ids):
        import jax
        jax.devices()
        if device_ids:
            ids = (ctypes.c_int64 * len(device_ids))(*device_ids)
            rc = lib.axon_start_nrt_profile(ids, len(device_ids))
        else:
            rc = lib.axon_start_nrt_profile(None, 0)
        if rc != 0:
            raise RuntimeError(f"axon_start_nrt_profile rc={rc}")
        try:
            yield
        finally:
            n = lib.axon_stop_nrt_profile(str(output_dir).encode())
            if n < 0:
                raise RuntimeError(f"axon_stop_nrt_profile rc={n}")

    mod = types.ModuleType("antenv.axon_hooks")
    mod.get_axon_ntff_profile_hook = lambda: _hook
    mod.set_axon_ntff_profile_hook = lambda h: None
    sys.modules["antenv.axon_hooks"] = mod
    import antenv
    antenv.axon_hooks = mod


_install_ntff_hook()

import concourse.bass as bass
import concourse.tile as tile
from concourse import bacc, mybir
import concourse.bass_utils as _bu
from concourse.bass_utils import run_bass_kernel_spmd

# Skip the remote artifact upload in the profile path (no bucket here).
_bu.upload_artifacts = lambda tmpdir: tmpdir

F32 = mybir.dt.float32
BF16 = mybir.dt.bfloat16

C_IN = 192          # input channels (dim)
C_QKV = 288         # qkv channels per core (half of 576)
C_ATTN = 96         # attn channels per core (half of 192)
NPX = 65536         # pixels per image
HP = 128            # attention h'-rows per core
WP = 256            # attention w' columns
CTOK = 576          # channels per token in the aliased view
SCALE = 8 ** (-0.5)

NT = 512            # matmul free-dim tile (pixels)
DW_TE_SLABS = 9     # of the 9 depthwise slabs, how many go to TensorE

Add = mybir.AluOpType.add
Mult = mybir.AluOpType.mult
Sub = mybir.AluOpType.subtract
AX = mybir.AxisListType.X

_CACHE = {}


def _shuffle_perm(block=4):
    """src pixel index for each output pixel of shuffle_data (per channel)."""
    h = w = 256
    idx = np.arange(h * w).reshape(1, 1, h, w)
    x = np.transpose(idx, (0, 2, 3, 1)).reshape(1, h * w, 1)
    x = x.reshape(1, block, h // block, block, w // block, 1)
    x = np.transpose(x, (0, 2, 4, 1, 3, 5)).reshape(1, h * w, 1)
    return x.reshape(h * w).copy()


def _shuffle_back_perm(block=4):
    h = w = 256
    idx = np.arange(h * w).reshape(1, 1, h, w)
    x = np.transpose(idx, (0, 2, 3, 1)).reshape(1, h * w, 1)
    x = x.reshape(1, h // block, w // block, block, block, 1)
    x = np.transpose(x, (0, 3, 1, 4, 2, 5)).reshape(1, h * w, 1)
    return x.reshape(h * w).copy()


def _emit_qkv(ctx, tc, nc, xs, wqkvT, qkv_dram, ppool):
    """qkv[288, 65536] = wqkvT.T @ xs  (bf16, K=192 as 128+64)."""
    wpool = ctx.enter_context(tc.tile_pool(name="qkv_w", bufs=1))
    xpool = ctx.enter_context(tc.tile_pool(name="qkv_x", bufs=3))
    opool = ctx.enter_context(tc.tile_pool(name="qkv_o", bufs=3))

    wq0 = wpool.tile([128, C_QKV], BF16, tag="wq0")
    wq1 = wpool.tile([64, C_QKV], BF16, tag="wq1")
    nc.sync.dma_start(wq0[:], wqkvT[0:128, :])
    nc.sync.dma_start(wq1[:], wqkvT[128:192, :])

    mchunks = [(0, 128), (128, 256), (256, 288)]
    for mi, (m0, m1) in enumerate(mchunks):
        mm = m1 - m0
        for n2 in range(NPX // 1024):
            x0 = xpool.tile([128, 1024], BF16, tag="x0")
            x1 = xpool.tile([64, 1024], BF16, tag="x1")
            nc.sync.dma_start(x0[:], xs[0:128, bass.ts(n2, 1024)])
            nc.sync.dma_start(x1[:], xs[128:192, bass.ts(n2, 1024)])
            for h in range(2):
                n = n2 * 2 + h
                ps = ppool.tile([128, NT], F32, tag=f"bank{mi}")
                nc.tensor.matmul(ps[:mm, :], wq0[:, m0:m1],
                                 x0[:, bass.ts(h, NT)], start=True, stop=False)
                nc.tensor.matmul(ps[:mm, :], wq1[:, m0:m1],
                                 x1[:, bass.ts(h, NT)], start=False, stop=True)
                ot = opool.tile([128, NT], BF16, tag=f"o{mi}")
                nc.vector.tensor_copy(ot[:mm, :], ps[:mm, :])
                nc.scalar.dma_start(qkv_dram[m0:m1, bass.ts(n, NT)], ot[:mm, :])


def _emit_dw(ctx, tc, nc, qkv_dram, wdw, wdiag, qkv_dw_dram, ppool):
    """3x3 depthwise conv, pad=1, on [288, 256, 256] (shuffled space).

    Channel-quarter packing: slab s covers channels [32s, 32s+32); SBUF
    partition p = 4*c_sub + qt holds image rows [64qt, 64qt+64) of channel
    32s + c_sub.  Two 32-row windows per slab.  Taps are flat-pixel shifts
    (+-1, +-256); w-column wrap errors are subtracted post-hoc; image
    top/bottom halo rows are zeroed.  Slabs go to TensorE (diag matmuls
    accumulated in PSUM) or DVE (9-op scalar_tensor_tensor chain with
    shifted copies tl/tr so every op keeps 4B alignment for 2x mode).
    """
    wpool = ctx.enter_context(tc.tile_pool(name="dw_w", bufs=1))
    ipool = ctx.enter_context(tc.tile_pool(name="dw_i", bufs=2))
    apool = ctx.enter_context(tc.tile_pool(name="dw_a", bufs=2))

    NIN = 34 * 256 + 4        # 2 guard + 34 rows + 2 guard
    NOUT = 32 * 256

    for s in range(9):
        c0 = 32 * s
        use_te = s < DW_TE_SLABS
        wt = wpool.tile([128, 9], F32, tag="wdw")
        nc.sync.dma_start(wt[:], wdw[s])
        wneg = wpool.tile([128, 9], F32, tag="wneg")
        nc.vector.tensor_scalar_mul(wneg[:], wt[:], -1.0)
        if use_te:
            wd = wpool.tile([128, 9 * 128], BF16, tag="wdiag")
            nc.sync.dma_start(
                wd[:].rearrange("p (t m) -> p t m", t=9),
                wdiag[s].rearrange("t k m -> k t m"))

        for w in range(2):
            it = ipool.tile([128, NIN], BF16, tag="in")
            # guards
            nc.vector.memset(it[:, 0:2], 0.0)
            nc.vector.memset(it[:, NIN - 2:NIN], 0.0)
            for qt in range(4):
                r_top = 64 * qt + 32 * w - 1          # first halo image row
                lo = max(0, r_top)
                hi = min(256, r_top + 34)
                dst0 = 2 + (lo - r_top) * 256
                nc.sync.dma_start(
                    it[32 * qt:32 * qt + 32, dst0:dst0 + (hi - lo) * 256],
                    qkv_dram[c0:c0 + 32, lo * 256:hi * 256])
                if lo > r_top:
                    nc.vector.memset(it[32 * qt:32 * qt + 32, 2:2 + 256], 0.0)
                if hi < r_top + 34:
                    nc.vector.memset(
                        it[32 * qt:32 * qt + 32, 2 + 33 * 256:2 + 34 * 256], 0.0)

            acc = apool.tile([128, NOUT], BF16, tag="acc")
            if use_te:
                for quad in range(4):
                    pss = []
                    for k in range(4):
                        psk = ppool.tile([128, NT], F32, tag=f"bank{k % 2 * 2 + k // 2}")
                        pss.append(psk)
                    for t9, (dh, dw2) in enumerate(
                            (dh, dw2) for dh in (-1, 0, 1) for dw2 in (-1, 0, 1)):
                        for k in range(4):
                            n = quad * 4 + k
                            off = 2 + 256 + n * NT + dh * 256 + dw2
                            nc.tensor.matmul(
                                pss[k][:], wd[:, bass.ts(t9, 128)],
                                it[:, off:off + NT],
                                start=(t9 == 0), stop=(t9 == 8))
                    for k in range(4):
                        nc.vector.tensor_copy(
                            acc[:, bass.ts(quad * 4 + k, NT)], pss[k][:])
            else:
                first = True
                for dh in (-1, 0, 1):
                    for dw2 in (-1, 0, 1):
                        t9 = (dh + 1) * 3 + (dw2 + 1)
                        sap = it[:, 2 + (dh + 1) * 256 + dw2:
                                 2 + (dh + 1) * 256 + dw2 + NOUT]
                        if first:
                            nc.vector.tensor_scalar_mul(
                                acc[:], sap, wt[:, t9:t9 + 1])
                            first = False
                        else:
                            nc.vector.scalar_tensor_tensor(
                                acc[:], sap, wt[:, t9:t9 + 1], acc[:],
                                op0=Mult, op1=Add)

            # subtract wrong w-wrap contributions at columns 0 and 255
            a3 = acc[:].rearrange("p (r w) -> p r w", w=256)
            i1 = it[:]
            for dh in (-1, 0, 1):
                tL = (dh + 1) * 3 + 0
                tR = (dh + 1) * 3 + 2
                # left col 0: tap (dh,-1) read it[1 + (dh+1)*256 + r*256]
                oL = 1 + (dh + 1) * 256
                srcL = i1[:, oL:oL + 31 * 256 + 1:256].unsqueeze(2)
                # right col 255: tap (dh,+1) read it[2+(dh+1)*256 + r*256 + 256]
                oR = 2 + (dh + 2) * 256
                srcR = i1[:, oR:oR + 31 * 256 + 1:256].unsqueeze(2)
                nc.vector.scalar_tensor_tensor(
                    a3[:, :, 0:1], srcL, wneg[:, tL:tL + 1],
                    a3[:, :, 0:1], op0=Mult, op1=Add)
                nc.vector.scalar_tensor_tensor(
                    a3[:, :, 255:256], srcR, wneg[:, tR:tR + 1],
                    a3[:, :, 255:256], op0=Mult, op1=Add)

            for qt in range(4):
                base = qt * 16384 + w * 8192
                nc.scalar.dma_start(
                    qkv_dw_dram[c0:c0 + 32, base:base + 8192],
                    acc[32 * qt:32 * qt + 32, :])


def _emit_attn(ctx, tc, nc, qkv_dw_dram, attn_dram):
    """2x2-window attention on the raw-reshape aliased view (bf16).

    Per-core flat qkv_dw [288*65536] == [128 h'-rows, 256 w', 576 c'].
    j in [0,128) (window column) rides the SBUF partition dim.
    """
    tpool = ctx.enter_context(tc.tile_pool(name="at_t", bufs=3))
    ppool = ctx.enter_context(tc.tile_pool(name="at_p", bufs=2))
    spool = ctx.enter_context(tc.tile_pool(name="at_s", bufs=3))
    opool = ctx.enter_context(tc.tile_pool(name="at_o", bufs=3))

    qv = qkv_dw_dram.rearrange("c p -> (c p)").rearrange(
        "(hh ww cc) -> hh ww cc", ww=WP, cc=CTOK)
    av = attn_dram.rearrange("c p -> (c p)").rearrange(
        "(hh ww cc) -> hh ww cc", ww=WP, cc=192)

    for g in range(HP // 2):
        T = tpool.tile([128, 4 * CTOK], BF16, tag="T")
        src = qv[2 * g:2 * g + 2].rearrange(
            "dh (j dw) c -> j dh dw c", dw=2)
        nc.sync.dma_start(
            T[:].rearrange("p (dh dw c) -> p dh dw c", dh=2, dw=2), src)
        t3 = T[:].rearrange("p (tok c) -> p tok c", tok=4)

        # QK^T products: P[p, (i j hd)]; dense innermost (2x mode)
        P = ppool.tile([128, 3072], BF16, tag="P")
        p3 = P[:].rearrange("p (i j hd) -> p i j hd", i=4, j=4)
        q_b = t3[:, :, 0:192].unsqueeze(2).broadcast_to([128, 4, 4, 192])
        k_b = t3[:, :, 192:384].unsqueeze(1).broadcast_to([128, 4, 4, 192])
        nc.vector.tensor_tensor(p3, q_b, k_b, op=Mult)

        # logits (f32) via GpSimd reduce over d
        L = spool.tile([128, 128], F32, tag="L")
        nc.vector.tensor_reduce(
            L[:], P[:].rearrange("p (ijh d) -> p ijh d", d=24), axis=AX, op=Add)

        # EE = exp(SCALE*L) expanded over d (ACT, step-0 input broadcast)
        EE = ppool.tile([128, 3072], BF16, tag="EE")
        nc.scalar.activation(
            EE[:].rearrange("p (ijh d) -> p ijh d", d=24),
            L[:].unsqueeze(2).broadcast_to([128, 128, 24]),
            mybir.ActivationFunctionType.Exp, scale=float(SCALE))

        # S[i,h] = sum_j exp; R = 1/S
        S = spool.tile([128, 32], F32, tag="S")
        ee4 = EE[:].rearrange("p (i j h d) -> p i j h d", i=4, j=4, h=8)
        nc.vector.tensor_reduce(
            S[:].rearrange("p (i h) -> p i h", i=4),
            ee4[:, :, :, :, 0].rearrange("p i j h -> p i h j"),
            axis=AX, op=Add)
        R = spool.tile([128, 32], F32, tag="R")
        nc.vector.reciprocal(R[:], S[:])

        # AV products: P2[p, (i j hd)] = EE * V (V broadcast over i; dense)
        P2 = ppool.tile([128, 3072], BF16, tag="P2")
        v_b = t3[:, :, 384:576].unsqueeze(1).broadcast_to([128, 4, 4, 192])
        nc.gpsimd.tensor_tensor(
            P2[:].rearrange("p (i j hd) -> p i j hd", i=4, j=4),
            EE[:].rearrange("p (i j hd) -> p i j hd", i=4, j=4), v_b, op=Mult)

        # sum over j (3 dense adds on GpSimd)
        p24 = P2[:].rearrange("p (i j hd) -> p i j hd", i=4, j=4)
        U = spool.tile([128, 768], BF16, tag="U")
        V2 = spool.tile([128, 768], BF16, tag="V2")
        u3 = U[:].rearrange("p (i hd) -> p i hd", i=4)
        v3 = V2[:].rearrange("p (i hd) -> p i hd", i=4)
        nc.vector.tensor_tensor(u3, p24[:, :, 0], p24[:, :, 1], op=Add)
        nc.vector.tensor_tensor(v3, p24[:, :, 2], p24[:, :, 3], op=Add)
        nc.gpsimd.tensor_tensor(u3, u3, v3, op=Add)

        # O = U * R (R broadcast over d), bf16 out
        O = opool.tile([128, 768], BF16, tag="O")
        r_b = R[:].rearrange("p (i h) -> p i h", i=4).unsqueeze(3).broadcast_to(
            [128, 4, 8, 24])
        nc.gpsimd.tensor_tensor(
            O[:].rearrange("p (i h d) -> p i h d", i=4, h=8),
            U[:].rearrange("p (i h d) -> p i h d", i=4, h=8), r_b, op=Mult)

        dst = av[2 * g:2 * g + 2].rearrange("dh (j dw) c -> j dh dw c", dw=2)
        nc.scalar.dma_start(
            dst, O[:].rearrange("p (dh dw c) -> p dh dw c", dh=2, dw=2))


def _emit_proj(ctx, tc, nc, attn_dram, wprojT, out_dram, ppool):
    """partial out[192, 65536] = wprojT.T @ attn[96, 65536] (bf16 in, f32 out)."""
    wpool = ctx.enter_context(tc.tile_pool(name="pj_w", bufs=1))
    xpool = ctx.enter_context(tc.tile_pool(name="pj_x", bufs=3))
    opool = ctx.enter_context(tc.tile_pool(name="pj_o", bufs=3))

    wp = wpool.tile([C_ATTN, 192], BF16, tag="wp")
    nc.sync.dma_start(wp[:], wprojT[:, :])

    for n in range(NPX // NT):
        xt = xpool.tile([C_ATTN, NT], BF16, tag="x")
        nc.sync.dma_start(xt[:], attn_dram[:, bass.ts(n, NT)])
        for mi, (m0, m1) in enumerate([(0, 128), (128, 192)]):
            mm = m1 - m0
            ps = ppool.tile([128, NT], F32, tag=f"bank{mi}")
            nc.tensor.matmul(ps[:mm, :], wp[:, m0:m1], xt[:],
                             start=True, stop=True)
            ot = opool.tile([128, NT], F32, tag=f"o{mi}")
            nc.vector.tensor_copy(ot[:mm, :], ps[:mm, :])
            nc.scalar.dma_start(out_dram[m0:m1, bass.ts(n, NT)], ot[:mm, :])


def _build():
    if "nc" in _CACHE:
        return _CACHE["nc"]
    nc = bacc.Bacc("TRN2", target_bir_lowering=False, debug=False,
                   num_devices=8)
    xs = nc.dram_tensor("xs", [C_IN, NPX], BF16, kind="ExternalInput").ap()
    wqkvT = nc.dram_tensor("wqkvT", [C_IN, C_QKV], BF16,
                           kind="ExternalInput").ap()
    wdw = nc.dram_tensor("wdw", [9, 128, 9], F32, kind="ExternalInput").ap()
    wdiag = nc.dram_tensor("wdiag", [9, 9, 128, 128], BF16,
                           kind="ExternalInput").ap()
    wprojT = nc.dram_tensor("wprojT", [C_ATTN, 192], BF16,
                            kind="ExternalInput").ap()
    out = nc.dram_tensor("out", [192, NPX], F32, kind="ExternalOutput").ap()

    qkv_dram = nc.dram_tensor("qkv_buf", [C_QKV, NPX], BF16).ap()
    qkv_dw_dram = nc.dram_tensor("qkv_dw_buf", [C_QKV, NPX], BF16).ap()
    attn_dram = nc.dram_tensor("attn_buf", [C_ATTN, NPX], BF16).ap()

    from contextlib import ExitStack
    with tile.TileContext(nc) as tc:
        with ExitStack() as ctx:
            psum = ctx.enter_context(
                tc.tile_pool(name="psum", bufs=2, space="PSUM"))
            _emit_qkv(ctx, tc, nc, xs, wqkvT, qkv_dram, psum)
            _emit_dw(ctx, tc, nc, qkv_dram, wdw, wdiag, qkv_dw_dram, psum)
            _emit_attn(ctx, tc, nc, qkv_dw_dram, attn_dram)
            _emit_proj(ctx, tc, nc, attn_dram, wprojT, out, psum)
    nc.compile()
    _CACHE["nc"] = nc
    return nc


def kernel(x, w_qkv, w_dw, w_proj, shuffle):
    import ml_dtypes
    bf = ml_dtypes.bfloat16
    x = np.asarray(x, dtype=np.float32)
    w_qkv = np.asarray(w_qkv, dtype=np.float32)
    w_dw = np.asarray(w_dw, dtype=np.float32)
    w_proj = np.asarray(w_proj, dtype=np.float32)
    do_shuffle = bool(int(np.asarray(shuffle)))

    B = x.shape[0]
    xf = x.reshape(B, C_IN, NPX)
    if do_shuffle:
        xf = xf[:, :, _shuffle_perm()]

    wq = w_qkv[:, :, 0, 0]                      # [576, 192]
    wqT = np.ascontiguousarray(wq.T)            # [192, 576]
    wdw_f = w_dw[:, 0].reshape(576, 9)          # [576, 9]
    wp = w_proj[:, :, 0, 0]                     # [192, 192]

    in_maps = []
    for b in range(B):
        for s in range(2):
            wdw_h = wdw_f[s * C_QKV:(s + 1) * C_QKV]      # [288, 9]
            # per-unit (channel-quarter packed) weights: [slab, 128, 9]
            wdw_u = np.stack([wdw_h[32 * sl + np.arange(128) % 32]
                              for sl in range(9)]).astype(np.float32)
            # diag matmul weights: [slab, tap, 128(K=unit), 128(M=unit)]
            wdiag = np.zeros((9, 9, 128, 128), dtype=bf)
            for sl in range(9):
                for t in range(9):
                    wdiag[sl, t][np.arange(128), np.arange(128)] = \
                        wdw_u[sl, :, t].astype(bf)
            in_maps.append({
                "xs": np.ascontiguousarray(xf[b]).astype(bf),
                "wqkvT": np.ascontiguousarray(
                    wqT[:, s * C_QKV:(s + 1) * C_QKV]).astype(bf),
                "wdw": wdw_u,
                "wdiag": wdiag,
                "wprojT": np.ascontiguousarray(
                    wp[:, s * C_ATTN:(s + 1) * C_ATTN].T).astype(bf),
            })

    nc = _build()
    res = run_bass_kernel_spmd(nc, in_maps, core_ids=list(range(8)),
                               trace=bool(int(os.environ.get("KERNEL_TRACE", "0"))))
    _CACHE["last_results"] = res

    outs = [res.results[i]["out"] for i in range(8)]
    of = np.stack([outs[2 * b].astype(np.float32) + outs[2 * b + 1].astype(np.float32)
                   for b in range(B)])
    if do_shuffle:
        of = of[:, :, _shuffle_back_perm()]
    return of.reshape(B, 192, 256, 256).astype(np.float32)


# revision 33
# speedup vs baseline: 1.1038x; 1.1038x over previous
"""Trainium2 Bass kernel for nn_Attention_31490700214694 (sparse_attention).

Pipeline (per batch, replicating the reference exactly, incl. the raw-reshape
aliasing in msa):
  x --shuffle--> xs --1x1 conv (192->576)--> qkv --dw3x3--> qkv_dw
     --window-attention (on the raw-reshape aliased view)--> attn
     --1x1 conv (192->192)--> out --shuffle_back--> result

Sharding: 8 cores = 4 batches x 2 channel-halves.  qkv channels [0:288) of a
batch alias exactly to attention rows h' in [0,128) (288*65536 == 128*147456),
and attention output rows h'' in [0,128) alias to attn channels [0:96).  So
each core runs the full middle pipeline independently on its half; the final
1x1 proj conv is computed as a partial sum over the core's 96 attn channels
and the two partials per batch are summed on the host.  No collectives.

v2: bf16 compute + intermediates; depthwise channel-quarter packing (zero
partition waste) with dense-AP STT chains (shifted-copy trick for DVE 2x
mode) on DVE and diag-matmul slabs on TensorE; attention with dense bf16
products (ACT exp-expansion) and GpSimd offloads.
"""

import os
import sys

import numpy as np

sys.path.insert(0, "/opt/trn_rl_repo")

def _install_ntff_hook():
    """Provide antenv.axon_hooks (missing in this image) so that
    run_bass_kernel_spmd(trace=True) can capture NTFF profiles."""
    import types
    import ctypes
    import contextlib

    if "antenv.axon_hooks" in sys.modules:
        return
    so_path = os.environ.get("PJRT_LIBRARY_PATH", "/opt/axon/libaxon_pjrt.so")
    try:
        lib = ctypes.CDLL(so_path)
    except OSError:
        return
    if not hasattr(lib, "axon_start_nrt_profile"):
        return
    lib.axon_start_nrt_profile.argtypes = [
        ctypes.POINTER(ctypes.c_int64), ctypes.c_size_t]
    lib.axon_start_nrt_profile.restype = ctypes.c_int64
    lib.axon_stop_nrt_profile.argtypes = [ctypes.c_char_p]
    lib.axon_stop_nrt_profile.restype = ctypes.c_int64

    @contextlib.contextmanager
    def _hook(output_dir, device_ids):
        import jax
        jax.devices()
        if device_ids:
            ids = (ctypes.c_int64 * len(device_ids))(*device_ids)
            rc = lib.axon_start_nrt_profile(ids, len(device_ids))
        else:
            rc = lib.axon_start_nrt_profile(None, 0)
        if rc != 0:
            raise RuntimeError(f"axon_start_nrt_profile rc={rc}")
        try:
            yield
        finally:
            n = lib.axon_stop_nrt_profile(str(output_dir).encode())
            if n < 0:
                raise RuntimeError(f"axon_stop_nrt_profile rc={n}")

    mod = types.ModuleType("antenv.axon_hooks")
    mod.get_axon_ntff_profile_hook = lambda: _hook
    mod.set_axon_ntff_profile_hook = lambda h: None
    sys.modules["antenv.axon_hooks"] = mod
    import antenv
    antenv.axon_hooks = mod


_install_ntff_hook()

import concourse.bass as bass
import concourse.tile as tile
from concourse import bacc, mybir
import concourse.bass_utils as _bu
from concourse.bass_utils import run_bass_kernel_spmd

# Skip the remote artifact upload in the profile path (no bucket here).
_bu.upload_artifacts = lambda tmpdir: tmpdir

F32 = mybir.dt.float32
BF16 = mybir.dt.bfloat16

C_IN = 192          # input channels (dim)
C_QKV = 288         # qkv channels per core (half of 576)
C_ATTN = 96         # attn channels per core (half of 192)
NPX = 65536         # pixels per image
HP = 128            # attention h'-rows per core
WP = 256            # attention w' columns
CTOK = 576          # channels per token in the aliased view
SCALE = 8 ** (-0.5)

NT = 512            # matmul free-dim tile (pixels)
DW_TE_SLABS = 9     # of the 9 depthwise slabs, how many go to TensorE

Add = mybir.AluOpType.add
Mult = mybir.AluOpType.mult
Sub = mybir.AluOpType.subtract
AX = mybir.AxisListType.X

_CACHE = {}


def _shuffle_perm(block=4):
    """src pixel index for each output pixel of shuffle_data (per channel)."""
    h = w = 256
    idx = np.arange(h * w).reshape(1, 1, h, w)
    x = np.transpose(idx, (0, 2, 3, 1)).reshape(1, h * w, 1)
    x = x.reshape(1, block, h // block, block, w // block, 1)
    x = np.transpose(x, (0, 2, 4, 1, 3, 5)).reshape(1, h * w, 1)
    return x.reshape(h * w).copy()


def _shuffle_back_perm(block=4):
    h = w = 256
    idx = np.arange(h * w).reshape(1, 1, h, w)
    x = np.transpose(idx, (0, 2, 3, 1)).reshape(1, h * w, 1)
    x = x.reshape(1, h // block, w // block, block, block, 1)
    x = np.transpose(x, (0, 3, 1, 4, 2, 5)).reshape(1, h * w, 1)
    return x.reshape(h * w).copy()


def _emit_qkv(ctx, tc, nc, xs, wqkvT, qkv_dram, ppool):
    """qkv[288, 65536] = wqkvT.T @ xs  (bf16, K=192 as 128+64)."""
    wpool = ctx.enter_context(tc.tile_pool(name="qkv_w", bufs=1))
    xpool = ctx.enter_context(tc.tile_pool(name="qkv_x", bufs=3))
    opool = ctx.enter_context(tc.tile_pool(name="qkv_o", bufs=3))

    wq0 = wpool.tile([128, C_QKV], BF16, tag="wq0")
    wq1 = wpool.tile([64, C_QKV], BF16, tag="wq1")
    nc.sync.dma_start(wq0[:], wqkvT[0:128, :])
    nc.sync.dma_start(wq1[:], wqkvT[128:192, :])

    mchunks = [(0, 128), (128, 256), (256, 288)]
    for n in range(NPX // NT):
        x0 = xpool.tile([128, NT], BF16, tag="x0")
        x1 = xpool.tile([64, NT], BF16, tag="x1")
        nc.sync.dma_start(x0[:], xs[0:128, bass.ts(n, NT)])
        nc.sync.dma_start(x1[:], xs[128:192, bass.ts(n, NT)])
        for mi, (m0, m1) in enumerate(mchunks):
            mm = m1 - m0
            ps = ppool.tile([128, NT], F32, tag=f"bank{mi}")
            nc.tensor.matmul(ps[:mm, :], wq0[:, m0:m1], x0[:],
                             start=True, stop=False)
            nc.tensor.matmul(ps[:mm, :], wq1[:, m0:m1], x1[:],
                             start=False, stop=True)
            ot = opool.tile([128, NT], BF16, tag=f"o{mi}")
            nc.vector.tensor_copy(ot[:mm, :], ps[:mm, :])
            nc.scalar.dma_start(qkv_dram[m0:m1, bass.ts(n, NT)], ot[:mm, :])


def _emit_dw(ctx, tc, nc, qkv_dram, wdw, wdiag, qkv_dw_dram, ppool):
    """3x3 depthwise conv, pad=1, on [288, 256, 256] (shuffled space).

    Channel-quarter packing: slab s covers channels [32s, 32s+32); SBUF
    partition p = 4*c_sub + qt holds image rows [64qt, 64qt+64) of channel
    32s + c_sub.  Two 32-row windows per slab.  Taps are flat-pixel shifts
    (+-1, +-256); w-column wrap errors are subtracted post-hoc; image
    top/bottom halo rows are zeroed.  Slabs go to TensorE (diag matmuls
    accumulated in PSUM) or DVE (9-op scalar_tensor_tensor chain with
    shifted copies tl/tr so every op keeps 4B alignment for 2x mode).
    """
    wpool = ctx.enter_context(tc.tile_pool(name="dw_w", bufs=1))
    ipool = ctx.enter_context(tc.tile_pool(name="dw_i", bufs=2))
    apool = ctx.enter_context(tc.tile_pool(name="dw_a", bufs=2))

    NIN = 34 * 256 + 4        # 2 guard + 34 rows + 2 guard
    NOUT = 32 * 256

    for s in range(9):
        c0 = 32 * s
        use_te = s < DW_TE_SLABS
        wt = wpool.tile([128, 9], F32, tag="wdw")
        nc.sync.dma_start(wt[:], wdw[s])
        wneg = wpool.tile([128, 9], F32, tag="wneg")
        nc.vector.tensor_scalar_mul(wneg[:], wt[:], -1.0)
        if use_te:
            wd = wpool.tile([128, 9 * 128], BF16, tag="wdiag")
            nc.sync.dma_start(
                wd[:].rearrange("p (t m) -> p t m", t=9),
                wdiag[s].rearrange("t k m -> k t m"))

        for w in range(2):
            it = ipool.tile([128, NIN], BF16, tag="in")
            # guards
            nc.vector.memset(it[:, 0:2], 0.0)
            nc.vector.memset(it[:, NIN - 2:NIN], 0.0)
            for qt in range(4):
                r_top = 64 * qt + 32 * w - 1          # first halo image row
                lo = max(0, r_top)
                hi = min(256, r_top + 34)
                dst0 = 2 + (lo - r_top) * 256
                nc.sync.dma_start(
                    it[32 * qt:32 * qt + 32, dst0:dst0 + (hi - lo) * 256],
                    qkv_dram[c0:c0 + 32, lo * 256:hi * 256])
                if lo > r_top:
                    nc.vector.memset(it[32 * qt:32 * qt + 32, 2:2 + 256], 0.0)
                if hi < r_top + 34:
                    nc.vector.memset(
                        it[32 * qt:32 * qt + 32, 2 + 33 * 256:2 + 34 * 256], 0.0)

            acc = apool.tile([128, NOUT], BF16, tag="acc")
            if use_te:
                for quad in range(4):
                    pss = []
                    for k in range(4):
                        psk = ppool.tile([128, NT], F32, tag=f"bank{k % 2 * 2 + k // 2}")
                        pss.append(psk)
                    for t9, (dh, dw2) in enumerate(
                            (dh, dw2) for dh in (-1, 0, 1) for dw2 in (-1, 0, 1)):
                        for k in range(4):
                            n = quad * 4 + k
                            off = 2 + 256 + n * NT + dh * 256 + dw2
                            nc.tensor.matmul(
                                pss[k][:], wd[:, bass.ts(t9, 128)],
                                it[:, off:off + NT],
                                start=(t9 == 0), stop=(t9 == 8))
                    for k in range(4):
                        nc.vector.tensor_copy(
                            acc[:, bass.ts(quad * 4 + k, NT)], pss[k][:])
            else:
                first = True
                for dh in (-1, 0, 1):
                    for dw2 in (-1, 0, 1):
                        t9 = (dh + 1) * 3 + (dw2 + 1)
                        sap = it[:, 2 + (dh + 1) * 256 + dw2:
                                 2 + (dh + 1) * 256 + dw2 + NOUT]
                        if first:
                            nc.vector.tensor_scalar_mul(
                                acc[:], sap, wt[:, t9:t9 + 1])
                            first = False
                        else:
                            nc.vector.scalar_tensor_tensor(
                                acc[:], sap, wt[:, t9:t9 + 1], acc[:],
                                op0=Mult, op1=Add)

            # subtract wrong w-wrap contributions at columns 0 and 255
            a3 = acc[:].rearrange("p (r w) -> p r w", w=256)
            i1 = it[:]
            for dh in (-1, 0, 1):
                tL = (dh + 1) * 3 + 0
                tR = (dh + 1) * 3 + 2
                # left col 0: tap (dh,-1) read it[1 + (dh+1)*256 + r*256]
                oL = 1 + (dh + 1) * 256
                srcL = i1[:, oL:oL + 31 * 256 + 1:256].unsqueeze(2)
                # right col 255: tap (dh,+1) read it[2+(dh+1)*256 + r*256 + 256]
                oR = 2 + (dh + 2) * 256
                srcR = i1[:, oR:oR + 31 * 256 + 1:256].unsqueeze(2)
                nc.vector.scalar_tensor_tensor(
                    a3[:, :, 0:1], srcL, wneg[:, tL:tL + 1],
                    a3[:, :, 0:1], op0=Mult, op1=Add)
                nc.vector.scalar_tensor_tensor(
                    a3[:, :, 255:256], srcR, wneg[:, tR:tR + 1],
                    a3[:, :, 255:256], op0=Mult, op1=Add)

            for qt in range(4):
                base = qt * 16384 + w * 8192
                nc.scalar.dma_start(
                    qkv_dw_dram[c0:c0 + 32, base:base + 8192],
                    acc[32 * qt:32 * qt + 32, :])


def _emit_attn(ctx, tc, nc, qkv_dw_dram, attn_dram):
    """2x2-window attention on the raw-reshape aliased view (bf16).

    Per-core flat qkv_dw [288*65536] == [128 h'-rows, 256 w', 576 c'].
    j in [0,128) (window column) rides the SBUF partition dim.
    """
    tpool = ctx.enter_context(tc.tile_pool(name="at_t", bufs=4))
    ppool = ctx.enter_context(tc.tile_pool(name="at_p", bufs=3))
    spool = ctx.enter_context(tc.tile_pool(name="at_s", bufs=3))
    opool = ctx.enter_context(tc.tile_pool(name="at_o", bufs=3))

    qv = qkv_dw_dram.rearrange("c p -> (c p)").rearrange(
        "(hh ww cc) -> hh ww cc", ww=WP, cc=CTOK)
    av = attn_dram.rearrange("c p -> (c p)").rearrange(
        "(hh ww cc) -> hh ww cc", ww=WP, cc=192)

    for g in range(HP // 2):
        T = tpool.tile([128, 4 * CTOK], BF16, tag="T")
        src = qv[2 * g:2 * g + 2].rearrange(
            "dh (j dw) c -> j dh dw c", dw=2)
        nc.sync.dma_start(
            T[:].rearrange("p (dh dw c) -> p dh dw c", dh=2, dw=2), src)
        t3 = T[:].rearrange("p (tok c) -> p tok c", tok=4)

        # QK^T products: P[p, (i j hd)]; dense innermost (2x mode)
        P = ppool.tile([128, 3072], BF16, tag="P")
        p3 = P[:].rearrange("p (i j hd) -> p i j hd", i=4, j=4)
        q_b = t3[:, :, 0:192].unsqueeze(2).broadcast_to([128, 4, 4, 192])
        k_b = t3[:, :, 192:384].unsqueeze(1).broadcast_to([128, 4, 4, 192])
        nc.vector.tensor_tensor(p3, q_b, k_b, op=Mult)

        # logits (f32) via GpSimd reduce over d
        L = spool.tile([128, 128], F32, tag="L")
        nc.vector.tensor_reduce(
            L[:], P[:].rearrange("p (ijh d) -> p ijh d", d=24), axis=AX, op=Add)

        # EE = exp(SCALE*L) expanded over d (ACT, step-0 input broadcast)
        EE = ppool.tile([128, 3072], BF16, tag="EE")
        nc.scalar.activation(
            EE[:].rearrange("p (ijh d) -> p ijh d", d=24),
            L[:].unsqueeze(2).broadcast_to([128, 128, 24]),
            mybir.ActivationFunctionType.Exp, scale=float(SCALE))

        # S[i,h] = sum_j exp; R = 1/S
        S = spool.tile([128, 32], F32, tag="S")
        ee4 = EE[:].rearrange("p (i j h d) -> p i j h d", i=4, j=4, h=8)
        nc.vector.tensor_reduce(
            S[:].rearrange("p (i h) -> p i h", i=4),
            ee4[:, :, :, :, 0].rearrange("p i j h -> p i h j"),
            axis=AX, op=Add)
        R = spool.tile([128, 32], F32, tag="R")
        nc.vector.reciprocal(R[:], S[:])

        # AV products: P2[p, (i j hd)] = EE * V (V broadcast over i; dense)
        P2 = ppool.tile([128, 3072], BF16, tag="P2")
        v_b = t3[:, :, 384:576].unsqueeze(1).broadcast_to([128, 4, 4, 192])
        nc.gpsimd.tensor_tensor(
            P2[:].rearrange("p (i j hd) -> p i j hd", i=4, j=4),
            EE[:].rearrange("p (i j hd) -> p i j hd", i=4, j=4), v_b, op=Mult)

        # sum over j (3 dense adds on GpSimd)
        p24 = P2[:].rearrange("p (i j hd) -> p i j hd", i=4, j=4)
        U = spool.tile([128, 768], BF16, tag="U")
        V2 = spool.tile([128, 768], BF16, tag="V2")
        u3 = U[:].rearrange("p (i hd) -> p i hd", i=4)
        v3 = V2[:].rearrange("p (i hd) -> p i hd", i=4)
        nc.vector.tensor_tensor(u3, p24[:, :, 0], p24[:, :, 1], op=Add)
        nc.vector.tensor_tensor(v3, p24[:, :, 2], p24[:, :, 3], op=Add)
        nc.gpsimd.tensor_tensor(u3, u3, v3, op=Add)

        # O = U * R (R broadcast over d), bf16 out
        O = opool.tile([128, 768], BF16, tag="O")
        r_b = R[:].rearrange("p (i h) -> p i h", i=4).unsqueeze(3).broadcast_to(
            [128, 4, 8, 24])
        nc.gpsimd.tensor_tensor(
            O[:].rearrange("p (i h d) -> p i h d", i=4, h=8),
            U[:].rearrange("p (i h d) -> p i h d", i=4, h=8), r_b, op=Mult)

        dst = av[2 * g:2 * g + 2].rearrange("dh (j dw) c -> j dh dw c", dw=2)
        nc.scalar.dma_start(
            dst, O[:].rearrange("p (dh dw c) -> p dh dw c", dh=2, dw=2))


def _emit_proj(ctx, tc, nc, attn_dram, wprojT, out_dram, ppool):
    """partial out[192, 65536] = wprojT.T @ attn[96, 65536] (bf16 in, f32 out)."""
    wpool = ctx.enter_context(tc.tile_pool(name="pj_w", bufs=1))
    xpool = ctx.enter_context(tc.tile_pool(name="pj_x", bufs=3))
    opool = ctx.enter_context(tc.tile_pool(name="pj_o", bufs=3))

    wp = wpool.tile([C_ATTN, 192], BF16, tag="wp")
    nc.sync.dma_start(wp[:], wprojT[:, :])

    for n in range(NPX // NT):
        xt = xpool.tile([C_ATTN, NT], BF16, tag="x")
        nc.sync.dma_start(xt[:], attn_dram[:, bass.ts(n, NT)])
        for mi, (m0, m1) in enumerate([(0, 128), (128, 192)]):
            mm = m1 - m0
            ps = ppool.tile([128, NT], F32, tag=f"bank{mi}")
            nc.tensor.matmul(ps[:mm, :], wp[:, m0:m1], xt[:],
                             start=True, stop=True)
            ot = opool.tile([128, NT], F32, tag=f"o{mi}")
            nc.vector.tensor_copy(ot[:mm, :], ps[:mm, :])
            nc.scalar.dma_start(out_dram[m0:m1, bass.ts(n, NT)], ot[:mm, :])


def _build():
    if "nc" in _CACHE:
        return _CACHE["nc"]
    nc = bacc.Bacc("TRN2", target_bir_lowering=False, debug=False,
                   num_devices=8)
    xs = nc.dram_tensor("xs", [C_IN, NPX], BF16, kind="ExternalInput").ap()
    wqkvT = nc.dram_tensor("wqkvT", [C_IN, C_QKV], BF16,
                           kind="ExternalInput").ap()
    wdw = nc.dram_tensor("wdw", [9, 128, 9], F32, kind="ExternalInput").ap()
    wdiag = nc.dram_tensor("wdiag", [9, 9, 128, 128], BF16,
                           kind="ExternalInput").ap()
    wprojT = nc.dram_tensor("wprojT", [C_ATTN, 192], BF16,
                            kind="ExternalInput").ap()
    out = nc.dram_tensor("out", [192, NPX], F32, kind="ExternalOutput").ap()

    qkv_dram = nc.dram_tensor("qkv_buf", [C_QKV, NPX], BF16).ap()
    qkv_dw_dram = nc.dram_tensor("qkv_dw_buf", [C_QKV, NPX], BF16).ap()
    attn_dram = nc.dram_tensor("attn_buf", [C_ATTN, NPX], BF16).ap()

    from contextlib import ExitStack
    with tile.TileContext(nc) as tc:
        with ExitStack() as ctx:
            psum = ctx.enter_context(
                tc.tile_pool(name="psum", bufs=2, space="PSUM"))
            _emit_qkv(ctx, tc, nc, xs, wqkvT, qkv_dram, psum)
            _emit_dw(ctx, tc, nc, qkv_dram, wdw, wdiag, qkv_dw_dram, psum)
            _emit_attn(ctx, tc, nc, qkv_dw_dram, attn_dram)
            _emit_proj(ctx, tc, nc, attn_dram, wprojT, out, psum)
    nc.compile()
    _CACHE["nc"] = nc
    return nc


def kernel(x, w_qkv, w_dw, w_proj, shuffle):
    import ml_dtypes
    bf = ml_dtypes.bfloat16
    x = np.asarray(x, dtype=np.float32)
    w_qkv = np.asarray(w_qkv, dtype=np.float32)
    w_dw = np.asarray(w_dw, dtype=np.float32)
    w_proj = np.asarray(w_proj, dtype=np.float32)
    do_shuffle = bool(int(np.asarray(shuffle)))

    B = x.shape[0]
    xf = x.reshape(B, C_IN, NPX)
    if do_shuffle:
        xf = xf[:, :, _shuffle_perm()]

    wq = w_qkv[:, :, 0, 0]                      # [576, 192]
    wqT = np.ascontiguousarray(wq.T)            # [192, 576]
    wdw_f = w_dw[:, 0].reshape(576, 9)          # [576, 9]
    wp = w_proj[:, :, 0, 0]                     # [192, 192]

    in_maps = []
    for b in range(B):
        for s in range(2):
            wdw_h = wdw_f[s * C_QKV:(s + 1) * C_QKV]      # [288, 9]
            # per-unit (channel-quarter packed) weights: [slab, 128, 9]
            wdw_u = np.stack([wdw_h[32 * sl + np.arange(128) % 32]
                              for sl in range(9)]).astype(np.float32)
            # diag matmul weights: [slab, tap, 128(K=unit), 128(M=unit)]
            wdiag = np.zeros((9, 9, 128, 128), dtype=bf)
            for sl in range(9):
                for t in range(9):
                    wdiag[sl, t][np.arange(128), np.arange(128)] = \
                        wdw_u[sl, :, t].astype(bf)
            in_maps.append({
                "xs": np.ascontiguousarray(xf[b]).astype(bf),
                "wqkvT": np.ascontiguousarray(
                    wqT[:, s * C_QKV:(s + 1) * C_QKV]).astype(bf),
                "wdw": wdw_u,
                "wdiag": wdiag,
                "wprojT": np.ascontiguousarray(
                    wp[:, s * C_ATTN:(s + 1) * C_ATTN].T).astype(bf),
            })

    nc = _build()
    res = run_bass_kernel_spmd(nc, in_maps, core_ids=list(range(8)),
                               trace=bool(int(os.environ.get("KERNEL_TRACE", "0"))))
    _CACHE["last_results"] = res

    outs = [res.results[i]["out"] for i in range(8)]
    of = np.stack([outs[2 * b].astype(np.float32) + outs[2 * b + 1].astype(np.float32)
                   for b in range(B)])
    if do_shuffle:
        of = of[:, :, _shuffle_back_perm()]
    return of.reshape(B, 192, 256, 256).astype(np.float32)


# revision 34
# speedup vs baseline: 1.1290x; 1.0229x over previous
"""Trainium2 Bass kernel for nn_Attention_31490700214694 (sparse_attention).

Pipeline (per batch, replicating the reference exactly, incl. the raw-reshape
aliasing in msa):
  x --shuffle--> xs --1x1 conv (192->576)--> qkv --dw3x3--> qkv_dw
     --window-attention (on the raw-reshape aliased view)--> attn
     --1x1 conv (192->192)--> out --shuffle_back--> result

Sharding: 8 cores = 4 batches x 2 channel-halves.  qkv channels [0:288) of a
batch alias exactly to attention rows h' in [0,128) (288*65536 == 128*147456),
and attention output rows h'' in [0,128) alias to attn channels [0:96).  So
each core runs the full middle pipeline independently on its half; the final
1x1 proj conv is computed as a partial sum over the core's 96 attn channels
and the two partials per batch are summed on the host.  No collectives.

v2: bf16 compute + intermediates; depthwise channel-quarter packing (zero
partition waste) with dense-AP STT chains (shifted-copy trick for DVE 2x
mode) on DVE and diag-matmul slabs on TensorE; attention with dense bf16
products (ACT exp-expansion) and GpSimd offloads.
"""

import os
import sys

import numpy as np

sys.path.insert(0, "/opt/trn_rl_repo")

def _install_ntff_hook():
    """Provide antenv.axon_hooks (missing in this image) so that
    run_bass_kernel_spmd(trace=True) can capture NTFF profiles."""
    import types
    import ctypes
    import contextlib

    if "antenv.axon_hooks" in sys.modules:
        return
    so_path = os.environ.get("PJRT_LIBRARY_PATH", "/opt/axon/libaxon_pjrt.so")
    try:
        lib = ctypes.CDLL(so_path)
    except OSError:
        return
    if not hasattr(lib, "axon_start_nrt_profile"):
        return
    lib.axon_start_nrt_profile.argtypes = [
        ctypes.POINTER(ctypes.c_int64), ctypes.c_size_t]
    lib.axon_start_nrt_profile.restype = ctypes.c_int64
    lib.axon_stop_nrt_profile.argtypes = [ctypes.c_char_p]
    lib.axon_stop_nrt_profile.restype = ctypes.c_int64

    @contextlib.contextmanager
    def _hook(output_dir, device_ids):
        import jax
        jax.devices()
        if device_ids:
            ids = (ctypes.c_int64 * len(device_ids))(*device_ids)
            rc = lib.axon_start_nrt_profile(ids, len(device_ids))
        else:
            rc = lib.axon_start_nrt_profile(None, 0)
        if rc != 0:
            raise RuntimeError(f"axon_start_nrt_profile rc={rc}")
        try:
            yield
        finally:
            n = lib.axon_stop_nrt_profile(str(output_dir).encode())
            if n < 0:
                raise RuntimeError(f"axon_stop_nrt_profile rc={n}")

    mod = types.ModuleType("antenv.axon_hooks")
    mod.get_axon_ntff_profile_hook = lambda: _hook
    mod.set_axon_ntff_profile_hook = lambda h: None
    sys.modules["antenv.axon_hooks"] = mod
    import antenv
    antenv.axon_hooks = mod


_install_ntff_hook()

import concourse.bass as bass
import concourse.tile as tile
from concourse import bacc, mybir
import concourse.bass_utils as _bu
from concourse.bass_utils import run_bass_kernel_spmd

# Skip the remote artifact upload in the profile path (no bucket here).
_bu.upload_artifacts = lambda tmpdir: tmpdir

F32 = mybir.dt.float32
BF16 = mybir.dt.bfloat16

C_IN = 192          # input channels (dim)
C_QKV = 288         # qkv channels per core (half of 576)
C_ATTN = 96         # attn channels per core (half of 192)
NPX = 65536         # pixels per image
HP = 128            # attention h'-rows per core
WP = 256            # attention w' columns
CTOK = 576          # channels per token in the aliased view
SCALE = 8 ** (-0.5)

NT = 512            # matmul free-dim tile (pixels)
DW_TE_SLABS = 9     # of the 9 depthwise slabs, how many go to TensorE

Add = mybir.AluOpType.add
Mult = mybir.AluOpType.mult
Sub = mybir.AluOpType.subtract
AX = mybir.AxisListType.X

_CACHE = {}


def _shuffle_perm(block=4):
    """src pixel index for each output pixel of shuffle_data (per channel)."""
    h = w = 256
    idx = np.arange(h * w).reshape(1, 1, h, w)
    x = np.transpose(idx, (0, 2, 3, 1)).reshape(1, h * w, 1)
    x = x.reshape(1, block, h // block, block, w // block, 1)
    x = np.transpose(x, (0, 2, 4, 1, 3, 5)).reshape(1, h * w, 1)
    return x.reshape(h * w).copy()


def _shuffle_back_perm(block=4):
    h = w = 256
    idx = np.arange(h * w).reshape(1, 1, h, w)
    x = np.transpose(idx, (0, 2, 3, 1)).reshape(1, h * w, 1)
    x = x.reshape(1, h // block, w // block, block, block, 1)
    x = np.transpose(x, (0, 3, 1, 4, 2, 5)).reshape(1, h * w, 1)
    return x.reshape(h * w).copy()


def _emit_qkv(ctx, tc, nc, xs, wqkvT, qkv_dram, ppool):
    """qkv[288, 65536] = wqkvT.T @ xs  (bf16, K=192 as 128+64)."""
    wpool = ctx.enter_context(tc.tile_pool(name="qkv_w", bufs=1))
    xpool = ctx.enter_context(tc.tile_pool(name="qkv_x", bufs=4))
    opool = ctx.enter_context(tc.tile_pool(name="qkv_o", bufs=4))

    wq0 = wpool.tile([128, C_QKV], BF16, tag="wq0")
    wq1 = wpool.tile([64, C_QKV], BF16, tag="wq1")
    nc.sync.dma_start(wq0[:], wqkvT[0:128, :])
    nc.sync.dma_start(wq1[:], wqkvT[128:192, :])

    mchunks = [(0, 128), (128, 256), (256, 288)]
    for n in range(NPX // NT):
        x0 = xpool.tile([128, NT], BF16, tag="x0")
        x1 = xpool.tile([64, NT], BF16, tag="x1")
        nc.sync.dma_start(x0[:], xs[0:128, bass.ts(n, NT)])
        nc.sync.dma_start(x1[:], xs[128:192, bass.ts(n, NT)])
        for mi, (m0, m1) in enumerate(mchunks):
            mm = m1 - m0
            ps = ppool.tile([128, NT], F32, tag=f"bank{mi}")
            nc.tensor.matmul(ps[:mm, :], wq0[:, m0:m1], x0[:],
                             start=True, stop=False)
            nc.tensor.matmul(ps[:mm, :], wq1[:, m0:m1], x1[:],
                             start=False, stop=True)
            ot = opool.tile([128, NT], BF16, tag=f"o{mi}")
            nc.vector.tensor_copy(ot[:mm, :], ps[:mm, :])
            nc.scalar.dma_start(qkv_dram[m0:m1, bass.ts(n, NT)], ot[:mm, :])


def _emit_dw(ctx, tc, nc, qkv_dram, wdw, wdiag, qkv_dw_dram, ppool):
    """3x3 depthwise conv, pad=1, on [288, 256, 256] (shuffled space).

    Channel-quarter packing: slab s covers channels [32s, 32s+32); SBUF
    partition p = 4*c_sub + qt holds image rows [64qt, 64qt+64) of channel
    32s + c_sub.  Two 32-row windows per slab.  Taps are flat-pixel shifts
    (+-1, +-256); w-column wrap errors are subtracted post-hoc; image
    top/bottom halo rows are zeroed.  Slabs go to TensorE (diag matmuls
    accumulated in PSUM) or DVE (9-op scalar_tensor_tensor chain with
    shifted copies tl/tr so every op keeps 4B alignment for 2x mode).
    """
    wpool = ctx.enter_context(tc.tile_pool(name="dw_w", bufs=1))
    ipool = ctx.enter_context(tc.tile_pool(name="dw_i", bufs=2))
    apool = ctx.enter_context(tc.tile_pool(name="dw_a", bufs=2))

    NIN = 34 * 256 + 4        # 2 guard + 34 rows + 2 guard
    NOUT = 32 * 256

    for s in range(9):
        c0 = 32 * s
        use_te = s < DW_TE_SLABS
        wt = wpool.tile([128, 9], F32, tag="wdw")
        nc.sync.dma_start(wt[:], wdw[s])
        wneg = wpool.tile([128, 9], F32, tag="wneg")
        nc.vector.tensor_scalar_mul(wneg[:], wt[:], -1.0)
        if use_te:
            wd = wpool.tile([128, 9 * 128], BF16, tag="wdiag")
            nc.sync.dma_start(
                wd[:].rearrange("p (t m) -> p t m", t=9),
                wdiag[s].rearrange("t k m -> k t m"))

        for w in range(2):
            it = ipool.tile([128, NIN], BF16, tag="in")
            # guards
            nc.vector.memset(it[:, 0:2], 0.0)
            nc.vector.memset(it[:, NIN - 2:NIN], 0.0)
            for qt in range(4):
                r_top = 64 * qt + 32 * w - 1          # first halo image row
                lo = max(0, r_top)
                hi = min(256, r_top + 34)
                dst0 = 2 + (lo - r_top) * 256
                nc.sync.dma_start(
                    it[32 * qt:32 * qt + 32, dst0:dst0 + (hi - lo) * 256],
                    qkv_dram[c0:c0 + 32, lo * 256:hi * 256])
                if lo > r_top:
                    nc.vector.memset(it[32 * qt:32 * qt + 32, 2:2 + 256], 0.0)
                if hi < r_top + 34:
                    nc.vector.memset(
                        it[32 * qt:32 * qt + 32, 2 + 33 * 256:2 + 34 * 256], 0.0)

            acc = apool.tile([128, NOUT], BF16, tag="acc")
            if use_te:
                for quad in range(4):
                    pss = []
                    for k in range(4):
                        psk = ppool.tile([128, NT], F32, tag=f"bank{k % 2 * 2 + k // 2}")
                        pss.append(psk)
                    for t9, (dh, dw2) in enumerate(
                            (dh, dw2) for dh in (-1, 0, 1) for dw2 in (-1, 0, 1)):
                        for k in range(4):
                            n = quad * 4 + k
                            off = 2 + 256 + n * NT + dh * 256 + dw2
                            nc.tensor.matmul(
                                pss[k][:], wd[:, bass.ts(t9, 128)],
                                it[:, off:off + NT],
                                start=(t9 == 0), stop=(t9 == 8))
                    for k in range(4):
                        nc.vector.tensor_copy(
                            acc[:, bass.ts(quad * 4 + k, NT)], pss[k][:])
            else:
                first = True
                for dh in (-1, 0, 1):
                    for dw2 in (-1, 0, 1):
                        t9 = (dh + 1) * 3 + (dw2 + 1)
                        sap = it[:, 2 + (dh + 1) * 256 + dw2:
                                 2 + (dh + 1) * 256 + dw2 + NOUT]
                        if first:
                            nc.vector.tensor_scalar_mul(
                                acc[:], sap, wt[:, t9:t9 + 1])
                            first = False
                        else:
                            nc.vector.scalar_tensor_tensor(
                                acc[:], sap, wt[:, t9:t9 + 1], acc[:],
                                op0=Mult, op1=Add)

            # subtract wrong w-wrap contributions at columns 0 and 255
            a3 = acc[:].rearrange("p (r w) -> p r w", w=256)
            i1 = it[:]
            for dh in (-1, 0, 1):
                tL = (dh + 1) * 3 + 0
                tR = (dh + 1) * 3 + 2
                # left col 0: tap (dh,-1) read it[1 + (dh+1)*256 + r*256]
                oL = 1 + (dh + 1) * 256
                srcL = i1[:, oL:oL + 31 * 256 + 1:256].unsqueeze(2)
                # right col 255: tap (dh,+1) read it[2+(dh+1)*256 + r*256 + 256]
                oR = 2 + (dh + 2) * 256
                srcR = i1[:, oR:oR + 31 * 256 + 1:256].unsqueeze(2)
                nc.vector.scalar_tensor_tensor(
                    a3[:, :, 0:1], srcL, wneg[:, tL:tL + 1],
                    a3[:, :, 0:1], op0=Mult, op1=Add)
                nc.vector.scalar_tensor_tensor(
                    a3[:, :, 255:256], srcR, wneg[:, tR:tR + 1],
                    a3[:, :, 255:256], op0=Mult, op1=Add)

            for qt in range(4):
                base = qt * 16384 + w * 8192
                nc.scalar.dma_start(
                    qkv_dw_dram[c0:c0 + 32, base:base + 8192],
                    acc[32 * qt:32 * qt + 32, :])


def _emit_attn(ctx, tc, nc, qkv_dw_dram, attn_dram):
    """2x2-window attention on the raw-reshape aliased view (bf16).

    Per-core flat qkv_dw [288*65536] == [128 h'-rows, 256 w', 576 c'].
    j in [0,128) (window column) rides the SBUF partition dim.
    """
    tpool = ctx.enter_context(tc.tile_pool(name="at_t", bufs=4))
    ppool = ctx.enter_context(tc.tile_pool(name="at_p", bufs=3))
    spool = ctx.enter_context(tc.tile_pool(name="at_s", bufs=3))
    opool = ctx.enter_context(tc.tile_pool(name="at_o", bufs=3))

    qv = qkv_dw_dram.rearrange("c p -> (c p)").rearrange(
        "(hh ww cc) -> hh ww cc", ww=WP, cc=CTOK)
    av = attn_dram.rearrange("c p -> (c p)").rearrange(
        "(hh ww cc) -> hh ww cc", ww=WP, cc=192)

    for g in range(HP // 2):
        T = tpool.tile([128, 4 * CTOK], BF16, tag="T")
        src = qv[2 * g:2 * g + 2].rearrange(
            "dh (j dw) c -> j dh dw c", dw=2)
        nc.sync.dma_start(
            T[:].rearrange("p (dh dw c) -> p dh dw c", dh=2, dw=2), src)
        t3 = T[:].rearrange("p (tok c) -> p tok c", tok=4)

        # QK^T products: P[p, (i j hd)]; dense innermost (2x mode)
        P = ppool.tile([128, 3072], BF16, tag="P")
        p3 = P[:].rearrange("p (i j hd) -> p i j hd", i=4, j=4)
        q_b = t3[:, :, 0:192].unsqueeze(2).broadcast_to([128, 4, 4, 192])
        k_b = t3[:, :, 192:384].unsqueeze(1).broadcast_to([128, 4, 4, 192])
        nc.vector.tensor_tensor(p3, q_b, k_b, op=Mult)

        # logits (f32) via GpSimd reduce over d
        L = spool.tile([128, 128], F32, tag="L")
        nc.vector.tensor_reduce(
            L[:], P[:].rearrange("p (ijh d) -> p ijh d", d=24), axis=AX, op=Add)

        # EE = exp(SCALE*L) expanded over d (ACT, step-0 input broadcast)
        EE = ppool.tile([128, 3072], BF16, tag="EE")
        nc.scalar.activation(
            EE[:].rearrange("p (ijh d) -> p ijh d", d=24),
            L[:].unsqueeze(2).broadcast_to([128, 128, 24]),
            mybir.ActivationFunctionType.Exp, scale=float(SCALE))

        # S[i,h] = sum_j exp; R = 1/S
        S = spool.tile([128, 32], F32, tag="S")
        ee4 = EE[:].rearrange("p (i j h d) -> p i j h d", i=4, j=4, h=8)
        nc.vector.tensor_reduce(
            S[:].rearrange("p (i h) -> p i h", i=4),
            ee4[:, :, :, :, 0].rearrange("p i j h -> p i h j"),
            axis=AX, op=Add)
        R = spool.tile([128, 32], F32, tag="R")
        nc.vector.reciprocal(R[:], S[:])

        # AV products: P2[p, (i j hd)] = EE * V (V broadcast over i; dense)
        P2 = ppool.tile([128, 3072], BF16, tag="P2")
        v_b = t3[:, :, 384:576].unsqueeze(1).broadcast_to([128, 4, 4, 192])
        nc.gpsimd.tensor_tensor(
            P2[:].rearrange("p (i j hd) -> p i j hd", i=4, j=4),
            EE[:].rearrange("p (i j hd) -> p i j hd", i=4, j=4), v_b, op=Mult)

        # sum over j (3 dense adds on GpSimd)
        p24 = P2[:].rearrange("p (i j hd) -> p i j hd", i=4, j=4)
        U = spool.tile([128, 768], BF16, tag="U")
        V2 = spool.tile([128, 768], BF16, tag="V2")
        u3 = U[:].rearrange("p (i hd) -> p i hd", i=4)
        v3 = V2[:].rearrange("p (i hd) -> p i hd", i=4)
        nc.vector.tensor_tensor(u3, p24[:, :, 0], p24[:, :, 1], op=Add)
        nc.vector.tensor_tensor(v3, p24[:, :, 2], p24[:, :, 3], op=Add)
        nc.gpsimd.tensor_tensor(u3, u3, v3, op=Add)

        # O = U * R (R broadcast over d), bf16 out
        O = opool.tile([128, 768], BF16, tag="O")
        r_b = R[:].rearrange("p (i h) -> p i h", i=4).unsqueeze(3).broadcast_to(
            [128, 4, 8, 24])
        nc.gpsimd.tensor_tensor(
            O[:].rearrange("p (i h d) -> p i h d", i=4, h=8),
            U[:].rearrange("p (i h d) -> p i h d", i=4, h=8), r_b, op=Mult)

        dst = av[2 * g:2 * g + 2].rearrange("dh (j dw) c -> j dh dw c", dw=2)
        nc.scalar.dma_start(
            dst, O[:].rearrange("p (dh dw c) -> p dh dw c", dh=2, dw=2))


def _emit_proj(ctx, tc, nc, attn_dram, wprojT, out_dram, ppool):
    """partial out[192, 65536] = wprojT.T @ attn[96, 65536] (bf16 in, f32 out)."""
    wpool = ctx.enter_context(tc.tile_pool(name="pj_w", bufs=1))
    xpool = ctx.enter_context(tc.tile_pool(name="pj_x", bufs=4))
    opool = ctx.enter_context(tc.tile_pool(name="pj_o", bufs=4))

    wp = wpool.tile([C_ATTN, 192], BF16, tag="wp")
    nc.sync.dma_start(wp[:], wprojT[:, :])

    for n in range(NPX // NT):
        xt = xpool.tile([C_ATTN, NT], BF16, tag="x")
        nc.sync.dma_start(xt[:], attn_dram[:, bass.ts(n, NT)])
        for mi, (m0, m1) in enumerate([(0, 128), (128, 192)]):
            mm = m1 - m0
            ps = ppool.tile([128, NT], F32, tag=f"bank{mi}")
            nc.tensor.matmul(ps[:mm, :], wp[:, m0:m1], xt[:],
                             start=True, stop=True)
            ot = opool.tile([128, NT], F32, tag=f"o{mi}")
            nc.vector.tensor_copy(ot[:mm, :], ps[:mm, :])
            nc.scalar.dma_start(out_dram[m0:m1, bass.ts(n, NT)], ot[:mm, :])


def _build():
    if "nc" in _CACHE:
        return _CACHE["nc"]
    nc = bacc.Bacc("TRN2", target_bir_lowering=False, debug=False,
                   num_devices=8)
    xs = nc.dram_tensor("xs", [C_IN, NPX], BF16, kind="ExternalInput").ap()
    wqkvT = nc.dram_tensor("wqkvT", [C_IN, C_QKV], BF16,
                           kind="ExternalInput").ap()
    wdw = nc.dram_tensor("wdw", [9, 128, 9], F32, kind="ExternalInput").ap()
    wdiag = nc.dram_tensor("wdiag", [9, 9, 128, 128], BF16,
                           kind="ExternalInput").ap()
    wprojT = nc.dram_tensor("wprojT", [C_ATTN, 192], BF16,
                            kind="ExternalInput").ap()
    out = nc.dram_tensor("out", [192, NPX], F32, kind="ExternalOutput").ap()

    qkv_dram = nc.dram_tensor("qkv_buf", [C_QKV, NPX], BF16).ap()
    qkv_dw_dram = nc.dram_tensor("qkv_dw_buf", [C_QKV, NPX], BF16).ap()
    attn_dram = nc.dram_tensor("attn_buf", [C_ATTN, NPX], BF16).ap()

    from contextlib import ExitStack
    with tile.TileContext(nc) as tc:
        with ExitStack() as ctx:
            psum = ctx.enter_context(
                tc.tile_pool(name="psum", bufs=2, space="PSUM"))
            _emit_qkv(ctx, tc, nc, xs, wqkvT, qkv_dram, psum)
            _emit_dw(ctx, tc, nc, qkv_dram, wdw, wdiag, qkv_dw_dram, psum)
            _emit_attn(ctx, tc, nc, qkv_dw_dram, attn_dram)
            _emit_proj(ctx, tc, nc, attn_dram, wprojT, out, psum)
    nc.compile()
    _CACHE["nc"] = nc
    return nc


def kernel(x, w_qkv, w_dw, w_proj, shuffle):
    import ml_dtypes
    bf = ml_dtypes.bfloat16
    x = np.asarray(x, dtype=np.float32)
    w_qkv = np.asarray(w_qkv, dtype=np.float32)
    w_dw = np.asarray(w_dw, dtype=np.float32)
    w_proj = np.asarray(w_proj, dtype=np.float32)
    do_shuffle = bool(int(np.asarray(shuffle)))

    B = x.shape[0]
    xf = x.reshape(B, C_IN, NPX)
    if do_shuffle:
        xf = xf[:, :, _shuffle_perm()]

    wq = w_qkv[:, :, 0, 0]                      # [576, 192]
    wqT = np.ascontiguousarray(wq.T)            # [192, 576]
    wdw_f = w_dw[:, 0].reshape(576, 9)          # [576, 9]
    wp = w_proj[:, :, 0, 0]                     # [192, 192]

    in_maps = []
    for b in range(B):
        for s in range(2):
            wdw_h = wdw_f[s * C_QKV:(s + 1) * C_QKV]      # [288, 9]
            # per-unit (channel-quarter packed) weights: [slab, 128, 9]
            wdw_u = np.stack([wdw_h[32 * sl + np.arange(128) % 32]
                              for sl in range(9)]).astype(np.float32)
            # diag matmul weights: [slab, tap, 128(K=unit), 128(M=unit)]
            wdiag = np.zeros((9, 9, 128, 128), dtype=bf)
            for sl in range(9):
                for t in range(9):
                    wdiag[sl, t][np.arange(128), np.arange(128)] = \
                        wdw_u[sl, :, t].astype(bf)
            in_maps.append({
                "xs": np.ascontiguousarray(xf[b]).astype(bf),
                "wqkvT": np.ascontiguousarray(
                    wqT[:, s * C_QKV:(s + 1) * C_QKV]).astype(bf),
                "wdw": wdw_u,
                "wdiag": wdiag,
                "wprojT": np.ascontiguousarray(
                    wp[:, s * C_ATTN:(s + 1) * C_ATTN].T).astype(bf),
            })

    nc = _build()
    res = run_bass_kernel_spmd(nc, in_maps, core_ids=list(range(8)),
                               trace=bool(int(os.environ.get("KERNEL_TRACE", "0"))))
    _CACHE["last_results"] = res

    outs = [res.results[i]["out"] for i in range(8)]
    of = np.stack([outs[2 * b].astype(np.float32) + outs[2 * b + 1].astype(np.float32)
                   for b in range(B)])
    if do_shuffle:
        of = of[:, :, _shuffle_back_perm()]
    return of.reshape(B, 192, 256, 256).astype(np.float32)


# revision 36
# speedup vs baseline: 1.1511x; 1.0196x over previous
"""Trainium2 Bass kernel for nn_Attention_31490700214694 (sparse_attention).

Pipeline (per batch, replicating the reference exactly, incl. the raw-reshape
aliasing in msa):
  x --shuffle--> xs --1x1 conv (192->576)--> qkv --dw3x3--> qkv_dw
     --window-attention (on the raw-reshape aliased view)--> attn
     --1x1 conv (192->192)--> out --shuffle_back--> result

Sharding: 8 cores = 4 batches x 2 channel-halves.  qkv channels [0:288) of a
batch alias exactly to attention rows h' in [0,128) (288*65536 == 128*147456),
and attention output rows h'' in [0,128) alias to attn channels [0:96).  So
each core runs the full middle pipeline independently on its half; the final
1x1 proj conv is computed as a partial sum over the core's 96 attn channels
and the two partials per batch are summed on the host.  No collectives.

v2: bf16 compute + intermediates; depthwise channel-quarter packing (zero
partition waste) with dense-AP STT chains (shifted-copy trick for DVE 2x
mode) on DVE and diag-matmul slabs on TensorE; attention with dense bf16
products (ACT exp-expansion) and GpSimd offloads.
"""

import os
import sys

import numpy as np

sys.path.insert(0, "/opt/trn_rl_repo")

def _install_ntff_hook():
    """Provide antenv.axon_hooks (missing in this image) so that
    run_bass_kernel_spmd(trace=True) can capture NTFF profiles."""
    import types
    import ctypes
    import contextlib

    if "antenv.axon_hooks" in sys.modules:
        return
    so_path = os.environ.get("PJRT_LIBRARY_PATH", "/opt/axon/libaxon_pjrt.so")
    try:
        lib = ctypes.CDLL(so_path)
    except OSError:
        return
    if not hasattr(lib, "axon_start_nrt_profile"):
        return
    lib.axon_start_nrt_profile.argtypes = [
        ctypes.POINTER(ctypes.c_int64), ctypes.c_size_t]
    lib.axon_start_nrt_profile.restype = ctypes.c_int64
    lib.axon_stop_nrt_profile.argtypes = [ctypes.c_char_p]
    lib.axon_stop_nrt_profile.restype = ctypes.c_int64

    @contextlib.contextmanager
    def _hook(output_dir, device_ids):
        import jax
        jax.devices()
        if device_ids:
            ids = (ctypes.c_int64 * len(device_ids))(*device_ids)
            rc = lib.axon_start_nrt_profile(ids, len(device_ids))
        else:
            rc = lib.axon_start_nrt_profile(None, 0)
        if rc != 0:
            raise RuntimeError(f"axon_start_nrt_profile rc={rc}")
        try:
            yield
        finally:
            n = lib.axon_stop_nrt_profile(str(output_dir).encode())
            if n < 0:
                raise RuntimeError(f"axon_stop_nrt_profile rc={n}")

    mod = types.ModuleType("antenv.axon_hooks")
    mod.get_axon_ntff_profile_hook = lambda: _hook
    mod.set_axon_ntff_profile_hook = lambda h: None
    sys.modules["antenv.axon_hooks"] = mod
    import antenv
    antenv.axon_hooks = mod


_install_ntff_hook()

import concourse.bass as bass
import concourse.tile as tile
from concourse import bacc, mybir
import concourse.bass_utils as _bu
from concourse.bass_utils import run_bass_kernel_spmd

# Skip the remote artifact upload in the profile path (no bucket here).
_bu.upload_artifacts = lambda tmpdir: tmpdir

F32 = mybir.dt.float32
BF16 = mybir.dt.bfloat16

C_IN = 192          # input channels (dim)
C_QKV = 288         # qkv channels per core (half of 576)
C_ATTN = 96         # attn channels per core (half of 192)
NPX = 65536         # pixels per image
HP = 128            # attention h'-rows per core
WP = 256            # attention w' columns
CTOK = 576          # channels per token in the aliased view
SCALE = 8 ** (-0.5)

NT = 512            # matmul free-dim tile (pixels)
DW_TE_SLABS = 9     # of the 9 depthwise slabs, how many go to TensorE

Add = mybir.AluOpType.add
Mult = mybir.AluOpType.mult
Sub = mybir.AluOpType.subtract
AX = mybir.AxisListType.X

_CACHE = {}


def _shuffle_perm(block=4):
    """src pixel index for each output pixel of shuffle_data (per channel)."""
    h = w = 256
    idx = np.arange(h * w).reshape(1, 1, h, w)
    x = np.transpose(idx, (0, 2, 3, 1)).reshape(1, h * w, 1)
    x = x.reshape(1, block, h // block, block, w // block, 1)
    x = np.transpose(x, (0, 2, 4, 1, 3, 5)).reshape(1, h * w, 1)
    return x.reshape(h * w).copy()


def _shuffle_back_perm(block=4):
    h = w = 256
    idx = np.arange(h * w).reshape(1, 1, h, w)
    x = np.transpose(idx, (0, 2, 3, 1)).reshape(1, h * w, 1)
    x = x.reshape(1, h // block, w // block, block, block, 1)
    x = np.transpose(x, (0, 3, 1, 4, 2, 5)).reshape(1, h * w, 1)
    return x.reshape(h * w).copy()


def _emit_qkv(ctx, tc, nc, xs, wqkvT, qkv_dram, ppool):
    """qkv[288, 65536] = wqkvT.T @ xs  (bf16, K=192 as 128+64)."""
    wpool = ctx.enter_context(tc.tile_pool(name="qkv_w", bufs=1))
    xpool = ctx.enter_context(tc.tile_pool(name="qkv_x", bufs=4))
    opool = ctx.enter_context(tc.tile_pool(name="qkv_o", bufs=4))

    wq0 = wpool.tile([128, C_QKV], BF16, tag="wq0")
    wq1 = wpool.tile([64, C_QKV], BF16, tag="wq1")
    nc.sync.dma_start(wq0[:], wqkvT[0:128, :])
    nc.sync.dma_start(wq1[:], wqkvT[128:192, :])

    mchunks = [(0, 128), (128, 256), (256, 288)]
    for n in range(NPX // NT):
        x0 = xpool.tile([128, NT], BF16, tag="x0")
        x1 = xpool.tile([64, NT], BF16, tag="x1")
        nc.sync.dma_start(x0[:], xs[0:128, bass.ts(n, NT)])
        nc.sync.dma_start(x1[:], xs[128:192, bass.ts(n, NT)])
        for mi, (m0, m1) in enumerate(mchunks):
            mm = m1 - m0
            ps = ppool.tile([128, NT], F32, tag=f"bank{mi}")
            nc.tensor.matmul(ps[:mm, :], wq0[:, m0:m1], x0[:],
                             start=True, stop=False)
            nc.tensor.matmul(ps[:mm, :], wq1[:, m0:m1], x1[:],
                             start=False, stop=True)
            ot = opool.tile([128, NT], BF16, tag=f"o{mi}")
            nc.vector.tensor_copy(ot[:mm, :], ps[:mm, :])
            nc.scalar.dma_start(qkv_dram[m0:m1, bass.ts(n, NT)], ot[:mm, :])


def _emit_dw(ctx, tc, nc, qkv_dram, wdw, wdiag, qkv_dw_dram, ppool):
    """3x3 depthwise conv, pad=1, on [288, 256, 256] (shuffled space).

    Channel-quarter packing: slab s covers channels [32s, 32s+32); SBUF
    partition p = 4*c_sub + qt holds image rows [64qt, 64qt+64) of channel
    32s + c_sub.  Two 32-row windows per slab.  Taps are flat-pixel shifts
    (+-1, +-256); w-column wrap errors are subtracted post-hoc; image
    top/bottom halo rows are zeroed.  Slabs go to TensorE (diag matmuls
    accumulated in PSUM) or DVE (9-op scalar_tensor_tensor chain with
    shifted copies tl/tr so every op keeps 4B alignment for 2x mode).
    """
    wpool = ctx.enter_context(tc.tile_pool(name="dw_w", bufs=2))
    ipool = ctx.enter_context(tc.tile_pool(name="dw_i", bufs=2))
    apool = ctx.enter_context(tc.tile_pool(name="dw_a", bufs=2))

    NIN = 34 * 256 + 4        # 2 guard + 34 rows + 2 guard
    NOUT = 32 * 256

    for s in range(9):
        c0 = 32 * s
        use_te = s < DW_TE_SLABS
        wt = wpool.tile([128, 9], F32, tag="wdw")
        nc.sync.dma_start(wt[:], wdw[s])
        wneg = wpool.tile([128, 9], F32, tag="wneg")
        nc.vector.tensor_scalar_mul(wneg[:], wt[:], -1.0)
        if use_te:
            wd = wpool.tile([128, 9 * 128], BF16, tag="wdiag")
            nc.sync.dma_start(
                wd[:].rearrange("p (t m) -> p t m", t=9),
                wdiag[s].rearrange("t k m -> k t m"))

        for w in range(2):
            it = ipool.tile([128, NIN], BF16, tag="in")
            # guards
            nc.vector.memset(it[:, 0:2], 0.0)
            nc.vector.memset(it[:, NIN - 2:NIN], 0.0)
            for qt in range(4):
                r_top = 64 * qt + 32 * w - 1          # first halo image row
                lo = max(0, r_top)
                hi = min(256, r_top + 34)
                dst0 = 2 + (lo - r_top) * 256
                nc.sync.dma_start(
                    it[32 * qt:32 * qt + 32, dst0:dst0 + (hi - lo) * 256],
                    qkv_dram[c0:c0 + 32, lo * 256:hi * 256])
                if lo > r_top:
                    nc.vector.memset(it[32 * qt:32 * qt + 32, 2:2 + 256], 0.0)
                if hi < r_top + 34:
                    nc.vector.memset(
                        it[32 * qt:32 * qt + 32, 2 + 33 * 256:2 + 34 * 256], 0.0)

            acc = apool.tile([128, NOUT], BF16, tag="acc")
            if use_te:
                for quad in range(4):
                    pss = []
                    for k in range(4):
                        psk = ppool.tile([128, NT], F32, tag=f"bank{k % 2 * 2 + k // 2}")
                        pss.append(psk)
                    for t9, (dh, dw2) in enumerate(
                            (dh, dw2) for dh in (-1, 0, 1) for dw2 in (-1, 0, 1)):
                        for k in range(4):
                            n = quad * 4 + k
                            off = 2 + 256 + n * NT + dh * 256 + dw2
                            nc.tensor.matmul(
                                pss[k][:], wd[:, bass.ts(t9, 128)],
                                it[:, off:off + NT],
                                start=(t9 == 0), stop=(t9 == 8))
                    for k in range(4):
                        nc.vector.tensor_copy(
                            acc[:, bass.ts(quad * 4 + k, NT)], pss[k][:])
            else:
                first = True
                for dh in (-1, 0, 1):
                    for dw2 in (-1, 0, 1):
                        t9 = (dh + 1) * 3 + (dw2 + 1)
                        sap = it[:, 2 + (dh + 1) * 256 + dw2:
                                 2 + (dh + 1) * 256 + dw2 + NOUT]
                        if first:
                            nc.vector.tensor_scalar_mul(
                                acc[:], sap, wt[:, t9:t9 + 1])
                            first = False
                        else:
                            nc.vector.scalar_tensor_tensor(
                                acc[:], sap, wt[:, t9:t9 + 1], acc[:],
                                op0=Mult, op1=Add)

            # subtract wrong w-wrap contributions at columns 0 and 255
            a3 = acc[:].rearrange("p (r w) -> p r w", w=256)
            i1 = it[:]
            for dh in (-1, 0, 1):
                tL = (dh + 1) * 3 + 0
                tR = (dh + 1) * 3 + 2
                # left col 0: tap (dh,-1) read it[1 + (dh+1)*256 + r*256]
                oL = 1 + (dh + 1) * 256
                srcL = i1[:, oL:oL + 31 * 256 + 1:256].unsqueeze(2)
                # right col 255: tap (dh,+1) read it[2+(dh+1)*256 + r*256 + 256]
                oR = 2 + (dh + 2) * 256
                srcR = i1[:, oR:oR + 31 * 256 + 1:256].unsqueeze(2)
                nc.vector.scalar_tensor_tensor(
                    a3[:, :, 0:1], srcL, wneg[:, tL:tL + 1],
                    a3[:, :, 0:1], op0=Mult, op1=Add)
                nc.vector.scalar_tensor_tensor(
                    a3[:, :, 255:256], srcR, wneg[:, tR:tR + 1],
                    a3[:, :, 255:256], op0=Mult, op1=Add)

            for qt in range(4):
                base = qt * 16384 + w * 8192
                nc.scalar.dma_start(
                    qkv_dw_dram[c0:c0 + 32, base:base + 8192],
                    acc[32 * qt:32 * qt + 32, :])


def _emit_attn(ctx, tc, nc, qkv_dw_dram, attn_dram):
    """2x2-window attention on the raw-reshape aliased view (bf16).

    Per-core flat qkv_dw [288*65536] == [128 h'-rows, 256 w', 576 c'].
    j in [0,128) (window column) rides the SBUF partition dim.
    """
    tpool = ctx.enter_context(tc.tile_pool(name="at_t", bufs=4))
    ppool = ctx.enter_context(tc.tile_pool(name="at_p", bufs=3))
    spool = ctx.enter_context(tc.tile_pool(name="at_s", bufs=4))
    opool = ctx.enter_context(tc.tile_pool(name="at_o", bufs=3))

    qv = qkv_dw_dram.rearrange("c p -> (c p)").rearrange(
        "(hh ww cc) -> hh ww cc", ww=WP, cc=CTOK)
    av = attn_dram.rearrange("c p -> (c p)").rearrange(
        "(hh ww cc) -> hh ww cc", ww=WP, cc=192)

    for g in range(HP // 2):
        T = tpool.tile([128, 4 * CTOK], BF16, tag="T")
        src = qv[2 * g:2 * g + 2].rearrange(
            "dh (j dw) c -> j dh dw c", dw=2)
        nc.sync.dma_start(
            T[:].rearrange("p (dh dw c) -> p dh dw c", dh=2, dw=2), src)
        t3 = T[:].rearrange("p (tok c) -> p tok c", tok=4)

        # QK^T products: P[p, (i j hd)]; dense innermost (2x mode)
        P = ppool.tile([128, 3072], BF16, tag="P")
        p3 = P[:].rearrange("p (i j hd) -> p i j hd", i=4, j=4)
        q_b = t3[:, :, 0:192].unsqueeze(2).broadcast_to([128, 4, 4, 192])
        k_b = t3[:, :, 192:384].unsqueeze(1).broadcast_to([128, 4, 4, 192])
        nc.vector.tensor_tensor(p3, q_b, k_b, op=Mult)

        # logits (f32) via GpSimd reduce over d
        L = spool.tile([128, 128], F32, tag="L")
        nc.vector.tensor_reduce(
            L[:], P[:].rearrange("p (ijh d) -> p ijh d", d=24), axis=AX, op=Add)

        # EE = exp(SCALE*L) expanded over d (ACT, step-0 input broadcast)
        EE = ppool.tile([128, 3072], BF16, tag="EE")
        nc.scalar.activation(
            EE[:].rearrange("p (ijh d) -> p ijh d", d=24),
            L[:].unsqueeze(2).broadcast_to([128, 128, 24]),
            mybir.ActivationFunctionType.Exp, scale=float(SCALE))

        # S[i,h] = sum_j exp; R = 1/S
        S = spool.tile([128, 32], F32, tag="S")
        ee4 = EE[:].rearrange("p (i j h d) -> p i j h d", i=4, j=4, h=8)
        nc.vector.tensor_reduce(
            S[:].rearrange("p (i h) -> p i h", i=4),
            ee4[:, :, :, :, 0].rearrange("p i j h -> p i h j"),
            axis=AX, op=Add)
        R = spool.tile([128, 32], F32, tag="R")
        nc.vector.reciprocal(R[:], S[:])

        # AV products: P2[p, (i j hd)] = EE * V (V broadcast over i; dense)
        P2 = ppool.tile([128, 3072], BF16, tag="P2")
        v_b = t3[:, :, 384:576].unsqueeze(1).broadcast_to([128, 4, 4, 192])
        nc.gpsimd.tensor_tensor(
            P2[:].rearrange("p (i j hd) -> p i j hd", i=4, j=4),
            EE[:].rearrange("p (i j hd) -> p i j hd", i=4, j=4), v_b, op=Mult)

        # sum over j (3 dense adds on GpSimd)
        p24 = P2[:].rearrange("p (i j hd) -> p i j hd", i=4, j=4)
        U = spool.tile([128, 768], BF16, tag="U")
        V2 = spool.tile([128, 768], BF16, tag="V2")
        u3 = U[:].rearrange("p (i hd) -> p i hd", i=4)
        v3 = V2[:].rearrange("p (i hd) -> p i hd", i=4)
        nc.vector.tensor_tensor(u3, p24[:, :, 0], p24[:, :, 1], op=Add)
        nc.vector.tensor_tensor(v3, p24[:, :, 2], p24[:, :, 3], op=Add)
        nc.gpsimd.tensor_tensor(u3, u3, v3, op=Add)

        # O = U * R (R broadcast over d), bf16 out
        O = opool.tile([128, 768], BF16, tag="O")
        r_b = R[:].rearrange("p (i h) -> p i h", i=4).unsqueeze(3).broadcast_to(
            [128, 4, 8, 24])
        nc.gpsimd.tensor_tensor(
            O[:].rearrange("p (i h d) -> p i h d", i=4, h=8),
            U[:].rearrange("p (i h d) -> p i h d", i=4, h=8), r_b, op=Mult)

        dst = av[2 * g:2 * g + 2].rearrange("dh (j dw) c -> j dh dw c", dw=2)
        nc.scalar.dma_start(
            dst, O[:].rearrange("p (dh dw c) -> p dh dw c", dh=2, dw=2))


def _emit_proj(ctx, tc, nc, attn_dram, wprojT, out_dram, ppool):
    """partial out[192, 65536] = wprojT.T @ attn[96, 65536] (bf16 in, f32 out)."""
    wpool = ctx.enter_context(tc.tile_pool(name="pj_w", bufs=1))
    xpool = ctx.enter_context(tc.tile_pool(name="pj_x", bufs=4))
    opool = ctx.enter_context(tc.tile_pool(name="pj_o", bufs=4))

    wp = wpool.tile([C_ATTN, 192], BF16, tag="wp")
    nc.sync.dma_start(wp[:], wprojT[:, :])

    for n in range(NPX // NT):
        xt = xpool.tile([C_ATTN, NT], BF16, tag="x")
        nc.sync.dma_start(xt[:], attn_dram[:, bass.ts(n, NT)])
        for mi, (m0, m1) in enumerate([(0, 128), (128, 192)]):
            mm = m1 - m0
            ps = ppool.tile([128, NT], F32, tag=f"bank{mi}")
            nc.tensor.matmul(ps[:mm, :], wp[:, m0:m1], xt[:],
                             start=True, stop=True)
            ot = opool.tile([128, NT], F32, tag=f"o{mi}")
            nc.vector.tensor_copy(ot[:mm, :], ps[:mm, :])
            nc.scalar.dma_start(out_dram[m0:m1, bass.ts(n, NT)], ot[:mm, :])


def _build():
    if "nc" in _CACHE:
        return _CACHE["nc"]
    nc = bacc.Bacc("TRN2", target_bir_lowering=False, debug=False,
                   num_devices=8)
    xs = nc.dram_tensor("xs", [C_IN, NPX], BF16, kind="ExternalInput").ap()
    wqkvT = nc.dram_tensor("wqkvT", [C_IN, C_QKV], BF16,
                           kind="ExternalInput").ap()
    wdw = nc.dram_tensor("wdw", [9, 128, 9], F32, kind="ExternalInput").ap()
    wdiag = nc.dram_tensor("wdiag", [9, 9, 128, 128], BF16,
                           kind="ExternalInput").ap()
    wprojT = nc.dram_tensor("wprojT", [C_ATTN, 192], BF16,
                            kind="ExternalInput").ap()
    out = nc.dram_tensor("out", [192, NPX], F32, kind="ExternalOutput").ap()

    qkv_dram = nc.dram_tensor("qkv_buf", [C_QKV, NPX], BF16).ap()
    qkv_dw_dram = nc.dram_tensor("qkv_dw_buf", [C_QKV, NPX], BF16).ap()
    attn_dram = nc.dram_tensor("attn_buf", [C_ATTN, NPX], BF16).ap()

    from contextlib import ExitStack
    with tile.TileContext(nc) as tc:
        with ExitStack() as ctx:
            psum = ctx.enter_context(
                tc.tile_pool(name="psum", bufs=2, space="PSUM"))
            _emit_qkv(ctx, tc, nc, xs, wqkvT, qkv_dram, psum)
            _emit_dw(ctx, tc, nc, qkv_dram, wdw, wdiag, qkv_dw_dram, psum)
            _emit_attn(ctx, tc, nc, qkv_dw_dram, attn_dram)
            _emit_proj(ctx, tc, nc, attn_dram, wprojT, out, psum)
    nc.compile()
    _CACHE["nc"] = nc
    return nc


def kernel(x, w_qkv, w_dw, w_proj, shuffle):
    import ml_dtypes
    bf = ml_dtypes.bfloat16
    x = np.asarray(x, dtype=np.float32)
    w_qkv = np.asarray(w_qkv, dtype=np.float32)
    w_dw = np.asarray(w_dw, dtype=np.float32)
    w_proj = np.asarray(w_proj, dtype=np.float32)
    do_shuffle = bool(int(np.asarray(shuffle)))

    B = x.shape[0]
    xf = x.reshape(B, C_IN, NPX)
    if do_shuffle:
        xf = xf[:, :, _shuffle_perm()]

    wq = w_qkv[:, :, 0, 0]                      # [576, 192]
    wqT = np.ascontiguousarray(wq.T)            # [192, 576]
    wdw_f = w_dw[:, 0].reshape(576, 9)          # [576, 9]
    wp = w_proj[:, :, 0, 0]                     # [192, 192]

    in_maps = []
    for b in range(B):
        for s in range(2):
            wdw_h = wdw_f[s * C_QKV:(s + 1) * C_QKV]      # [288, 9]
            # per-unit (channel-quarter packed) weights: [slab, 128, 9]
            wdw_u = np.stack([wdw_h[32 * sl + np.arange(128) % 32]
                              for sl in range(9)]).astype(np.float32)
            # diag matmul weights: [slab, tap, 128(K=unit), 128(M=unit)]
            wdiag = np.zeros((9, 9, 128, 128), dtype=bf)
            for sl in range(9):
                for t in range(9):
                    wdiag[sl, t][np.arange(128), np.arange(128)] = \
                        wdw_u[sl, :, t].astype(bf)
            in_maps.append({
                "xs": np.ascontiguousarray(xf[b]).astype(bf),
                "wqkvT": np.ascontiguousarray(
                    wqT[:, s * C_QKV:(s + 1) * C_QKV]).astype(bf),
                "wdw": wdw_u,
                "wdiag": wdiag,
                "wprojT": np.ascontiguousarray(
                    wp[:, s * C_ATTN:(s + 1) * C_ATTN].T).astype(bf),
            })

    nc = _build()
    res = run_bass_kernel_spmd(nc, in_maps, core_ids=list(range(8)),
                               trace=bool(int(os.environ.get("KERNEL_TRACE", "0"))))
    _CACHE["last_results"] = res

    outs = [res.results[i]["out"] for i in range(8)]
    of = np.stack([outs[2 * b].astype(np.float32) + outs[2 * b + 1].astype(np.float32)
                   for b in range(B)])
    if do_shuffle:
        of = of[:, :, _shuffle_back_perm()]
    return of.reshape(B, 192, 256, 256).astype(np.float32)
